# revision 17
# baseline (speedup 1.0000x reference)
"""Trainium2 Bass kernel for the trajectory-decoder LSTM problem.

Math (mirrors the reference, with algebraic folds):
  dec_inp feeds gates only through W_emb; W_sp/W_emb/W_hp collapse:
    W_es = W_emb @ W_sp            [4H, 2]
    gates_t = zx@W_zx.T + bias + r_{t-1}@W_es.T + h_{t-1}@W_hh.T
  For t>=1, r_{t-1} = h_{t-1}@W_hp.T + b_hp, so with
    W_hh' = W_hh + W_es @ W_hp,  bias1 = b_ih + b_hh + W_emb@b_sp + W_es@b_hp
  every step becomes uniform:  gates_t = zx@W_zx.T + bias1 + h_{t-1}@W_hh'.T
  Step 0, rewritten against the SAME stationary W_hh' (so the recurrence
  weights are reused):
    gates_0 = zxp + bias1 + h_init@W_hh'.T + psi@W_es.T
    psi = (lpr - b_hp) - h_init@W_hp.T            [B, 2]
  and after sigmoid_0 the psi term is removed again (gates_1.. use only
  dh@W_hh'.T deltas on the resident PSUM accumulator).
  `last_pos` is dead code (output is just the stacked rel_pos).

Device strategy (pure data-parallel over 8 cores, 4096 batch each):
  - batch on the free dim, features on partitions
  - per 256-batch wave, the 4 gate pre-activations live RESIDENT in one
    2-bank PSUM tile; each step the PE accumulates (h_t - h_{t-1}) @ W_hh'.T
    into it (start=False).  The two bank-open matmuls are K=6: rows 0-1
    inject bias1 via a 0/1 indicator, rows 2-5 inject psi@W_es.T via
    sel-masked psi rows (psi replicated into partition rows 2-5 of the
    moving tile by two tiny SBUF->SBUF DMAs).  psi itself is computed by
    the PE (h_init @ -W_hp.T accumulated with I2 @ (lpr-b_hp)) into the
    wave's own (not yet opened) gates bank.
  - contraction K=1056 exact (no 1152 pad): 8 full K-tiles plus one K=32
    tile evaluated with row-group packing (tile_position): 4 gate tiles
    concurrently for zxp, j-tile pairs for mlp1.  The K=32 zx rows are
    replicated across the 4 partition row-groups host-side.
  - the g-gate weights are doubled on host so ONE sigmoid over all 4 banks
    yields sig(i),sig(f),sig(o),sig(2g); tanh(g)=2*sig(2g)-1 folds into the
    DVE scalar_tensor_tensor ops for m1 and c.
  - per-step elementwise: sigmoid (ACT), then m1/m2/c and h/dh as
    back-to-back DVE ops; tanh on ACT.
  - rel_pos: h-stationary matmuls (batch on out partitions, N=2) into
    small scratch PSUM tiles, copied out via ACT+DVE, one contiguous
    partition-major DMA per wave.
  - waves are software-pipelined on a virtual timeline: every op gets
    an estimated execution time tau and ops are emitted sorted by tau,
    so each engine's in-order queue matches true readiness order.
    Chains step every L=4.6us; waves start every WS=22.4us (PE-
    feasibility bound).  Filler matmuls (mlp/init) are chopped into
    <=0.7us chunks; weight DMAs go on the idle GpSimd queue.
  - all matmul operands fp16 (full PE rate), PSUM accumulation fp32;
    c kept in fp16, b_hp added on host after gather.
"""

import numpy as np

B = 32768
NCORES = 8
BC = B // NCORES          # 4096 batch per core
WAVE = 256                # batch per recurrence chain (2 PSUM banks of gates)
NW = BC // WAVE           # 16 waves
PAIR = 2 * WAVE           # phase-A (mlp) runs at N=512 across wave pairs
T = 12                    # decode steps
H = 128
G4 = 4 * H                # 512 gate features
ZX = 1056
KT = 9                    # 8 full K-tiles + one packed K=32 tile
KP = KT * 128
MLP = 1024
EMB = 64

_cache = {}


def _build_nc():
    import concourse.bass as bass
    import concourse.bacc as bacc
    import concourse.mybir as mybir
    import concourse.tile as tile
    from concourse.bass import ts

    f16 = mybir.dt.float16
    f32 = mybir.dt.float32
    AF = mybir.ActivationFunctionType
    OP = mybir.AluOpType

    nc = bacc.Bacc("TRN2", target_bir_lowering=False)

    zxT = nc.dram_tensor("zxT", [KP, BC], f16, kind="ExternalInput")
    lprT = nc.dram_tensor("lprT", [2, BC], f16, kind="ExternalInput")
    w1t = nc.dram_tensor("w1t", [128, KT, MLP], f16, kind="ExternalInput")
    wzxt = nc.dram_tensor("wzxt", [128, KT, G4], f16, kind="ExternalInput")
    w2t = nc.dram_tensor("w2t", [128, 8, H], f16, kind="ExternalInput")
    whht = nc.dram_tensor("whht", [128, G4], f16, kind="ExternalInput")
    whpt = nc.dram_tensor("whpt", [128, 2], f16, kind="ExternalInput")
    whh0 = nc.dram_tensor("whh0", [128, G4], f16, kind="ExternalInput")
    whhn = nc.dram_tensor("whhn", [128, G4], f16, kind="ExternalInput")
    lhsT66 = nc.dram_tensor("lhsT66", [66, 2 * 128], f16, kind="ExternalInput")
    zzm = nc.dram_tensor("zzm", [66, NW * 512], f16, kind="ExternalInput")
    wes2 = nc.dram_tensor("wes2", [2, G4], f16, kind="ExternalInput")
    k32 = nc.dram_tensor("k32", [2, G4], f16, kind="ExternalInput")
    b1 = nc.dram_tensor("b1", [128, 8], f32, kind="ExternalInput")
    b2 = nc.dram_tensor("b2", [128, 1], f32, kind="ExternalInput")
    # per wave: [partition=batch, (blk, t, j)] — partition-major so the
    # rel output DMA is one clean 192B-per-partition descriptor
    pred = nc.dram_tensor("pred", [NW, 128, 4 * T], f32, kind="ExternalOutput")

    with tile.TileContext(nc) as tc:
        with (
            tc.tile_pool(name="consts", bufs=1) as cpool,
            tc.tile_pool(name="zx", bufs=2) as zxpool,
            tc.tile_pool(name="h1", bufs=2) as h1pool,
            tc.tile_pool(name="hc", bufs=10) as hcpool,
            tc.tile_pool(name="acts", bufs=6) as apool,
            tc.tile_pool(name="outs", bufs=3) as opool,
            tc.tile_pool(name="scrps", bufs=2, space="PSUM") as scrpool,
            tc.tile_pool(name="gateps", bufs=3, space="PSUM") as gatepool,
        ):
            # ---- constants: small ones loaded upfront; the big mlp/zxp
            # weights are DMA'd in chunks as tau-events ----
            w1t_s = cpool.tile([128, KT, MLP], f16)
            wzxt_s = cpool.tile([128, KT, G4], f16)
            w2t_s = cpool.tile([128, 8, H], f16)
            whht_s = cpool.tile([128, G4], f16)
            nc.sync.dma_start(whht_s[:], whht[:])
            whpt_s = cpool.tile([128, 2], f16)
            nc.sync.dma_start(whpt_s[:], whpt[:])
            whh0_s = cpool.tile([128, G4], f16)
            nc.sync.dma_start(whh0_s[:], whh0[:])
            whhn_s = cpool.tile([128, G4], f16)
            nc.sync.dma_start(whhn_s[:], whhn[:])
            lhsT66_s = cpool.tile([66, 2 * 128], f16)
            nc.sync.dma_start(lhsT66_s[:], lhsT66[:])
            zzm_s = cpool.tile([66, NW * 512], f16)
            wes2_s = cpool.tile([2, G4], f16)
            nc.sync.dma_start(wes2_s[:], wes2[:])
            k32_s = cpool.tile([2, G4], f16)
            nc.sync.dma_start(k32_s[:], k32[:])
            b1_s = cpool.tile([128, 8], f32)
            nc.sync.dma_start(b1_s[:], b1[:])
            b2_s = cpool.tile([128, 1], f32)
            nc.sync.dma_start(b2_s[:], b2[:])
            lpr_s = cpool.tile([2, BC], f16)
            nc.sync.dma_start(lpr_s[:], lprT[:])

            zxT_v = zxT.rearrange("(k p) b -> p k b", p=128)

            # Virtual-timeline schedule (see module docstring).
            NP = NW // 2
            state = [dict() for _ in range(NW)]
            pair_state = [dict() for _ in range(NP)]
            events = []

            L = 4600
            WS = 23400

            def ev(tau, fn):
                events.append((tau, len(events), fn))

            def mk_w1t_dma(jh):
                def fn():
                    nc.gpsimd.dma_start(
                        w1t_s[:, :, ts(jh, 256)], w1t[:, :, ts(jh, 256)]
                    )
                return fn

            def mk_wzxt_dma(half):
                def fn():
                    nc.gpsimd.dma_start(
                        wzxt_s[:, :, ts(half, 256)], wzxt[:, :, ts(half, 256)]
                    )
                return fn

            def mk_w2t_dma():
                def fn():
                    nc.sync.dma_start(w2t_s[:], w2t[:])
                return fn

            def mk_zxw(p):
                def fn():
                    st = pair_state[p]
                    zxw = zxpool.tile([128, KT, PAIR], f16, tag="zxw", name="zxw")
                    nc.gpsimd.dma_start(zxw[:], zxT_v[:, :, ts(p, PAIR)])
                    st["zxw"] = zxw
                    st["h1"] = h1pool.tile([128, 8, PAIR], f16, tag="h1", name="h1")
                return fn

            def mk_mlp1_mm(p, j, k0):
                def fn():
                    st = pair_state[p]
                    key = "ps%d" % (j % 2)
                    if k0 == 0:
                        st[key] = scrpool.tile([128, PAIR], f32, tag="scratch", name="ps")
                    for k in range(k0, min(k0 + 3, KT)):
                        nc.tensor.matmul(
                            st[key][:], w1t_s[:, k, ts(j, 128)], st["zxw"][:, k, :],
                            start=(k == 0), stop=(k == KT - 1),
                            skip_group_check=True,
                        )
                return fn

            # K=32 tail tile: j-pair 2-packed on PE row groups 0 / 32
            def mk_mlp1_pk(p, je):
                def fn():
                    st = pair_state[p]
                    nc.tensor.matmul(
                        st["ps0"][:], w1t_s[0:32, 8, ts(je, 128)],
                        st["zxw"][0:32, 8, :],
                        start=False, stop=True, skip_group_check=True,
                    )
                    nc.tensor.matmul(
                        st["ps1"][:], w1t_s[32:64, 8, ts(je + 1, 128)],
                        st["zxw"][32:64, 8, :],
                        start=False, stop=True, skip_group_check=True,
                    )
                return fn

            def mk_mlp1_ts(p, j):
                def fn():
                    st = pair_state[p]
                    key = "ps%d" % (j % 2)
                    nc.vector.tensor_scalar(
                        st["h1"][:, j, :], st[key][:], b1_s[:, j : j + 1],
                        0.0, OP.add, OP.max,
                    )
                return fn

            def mk_mlp2_mm(p, j0, nj):
                def fn():
                    st = pair_state[p]
                    if j0 == 0:
                        st["ps0"] = scrpool.tile([128, PAIR], f32, tag="scratch", name="ps")
                    for j in range(j0, j0 + nj):
                        nc.tensor.matmul(
                            st["ps0"][:], w2t_s[:, j, :], st["h1"][:, j, :],
                            start=(j == 0), stop=(j == 7), skip_group_check=True,
                        )
                return fn

            def mk_mlp2_ts(p):
                def fn():
                    st = pair_state[p]
                    hi = h1pool.tile([128, PAIR], f16, tag="hinit", name="hinit")
                    nc.vector.tensor_scalar(
                        hi[:], st["ps0"][:], b2_s[:, 0:1], 0.0, OP.add, OP.max
                    )
                    st["h_init"] = hi
                return fn

            # bank-open: K=66 matmul injects bias1 (indicator rows 0-1)
            # plus the z-dims' zxp contribution (sel-masked z rows 2-65),
            # and clears has_written for the whole bank
            def mk_open(w):
                def fn():
                    st = state[w]
                    gates = gatepool.tile([128, 4 * WAVE], f32, tag="gates", name="gates")
                    st["gates"] = gates
                    for bk in range(2):
                        nc.tensor.matmul(
                            gates[:, ts(bk, 2 * WAVE)],
                            lhsT66_s[:, ts(bk, 128)], zzm_s[:, ts(w, 512)],
                            start=True, stop=False, skip_group_check=True,
                        )
                return fn

            # gates_0 = zxp + bias + h_init@W_hh.T + (lpr-b_hp)@W_es.T
            def mk_init_wes(w):
                def fn():
                    st = state[w]
                    for g in range(4):
                        nc.tensor.matmul(
                            st["gates"][:, ts(g, WAVE)], wes2_s[:, ts(g, 128)],
                            lpr_s[:, ts(w, WAVE)],
                            start=False, stop=False, skip_group_check=True,
                        )
                return fn

            def mk_init_hh(w):
                def fn():
                    st = state[w]
                    st["h_prev"] = pair_state[w // 2]["h_init"][:, ts(w % 2, WAVE)]
                    for g in range(4):
                        nc.tensor.matmul(
                            st["gates"][:, ts(g, WAVE)], whh0_s[:, ts(g, 128)],
                            st["h_prev"][:],
                            start=False, stop=False, skip_group_check=True,
                        )
                return fn

            # step-0 removal: gates += -h_init@W_hh.T - (lpr-b_hp)@W_es.T
            def mk_t0rm(w):
                def fn():
                    st = state[w]
                    for g in range(4):
                        gp = st["gates"][:, ts(g, WAVE)]
                        nc.tensor.matmul(
                            gp[:], whhn_s[:, ts(g, 128)], st["h_prev"][:],
                            start=False, stop=False, skip_group_check=True,
                        )
                        nc.tensor.matmul(
                            gp[:], k32_s[:, ts(g, 128)], lpr_s[:, ts(w, WAVE)],
                            start=False, stop=False, skip_group_check=True,
                        )
                return fn

            def mk_zxp(w, g):
                def fn():
                    st = state[w]
                    pst = pair_state[w // 2]
                    hs = ts(w % 2, WAVE)
                    gp = st["gates"][:, ts(g, WAVE)]
                    for k in range(8):
                        nc.tensor.matmul(
                            gp[:], wzxt_s[:, k, ts(g, 128)], pst["zxw"][:, k, hs],
                            start=False, stop=False, skip_group_check=True,
                        )
                return fn

            # K=32 tail tile: all 4 gates concurrently on row groups 0..96
            def mk_zxp_pk(w):
                def fn():
                    st = state[w]
                    pst = pair_state[w // 2]
                    hs = ts(w % 2, WAVE)
                    for g in range(4):
                        nc.tensor.matmul(
                            st["gates"][:, ts(g, WAVE)],
                            wzxt_s[32 * g : 32 * g + 32, 8, ts(g, 128)],
                            pst["zxw"][32 * g : 32 * g + 32, 8, hs],
                            start=False, stop=False, skip_group_check=True,
                            tile_position=(32 * g, 0),
                        )
                return fn

            def mk_sig(w, t):
                def fn():
                    st = state[w]
                    sig = apool.tile([128, 4 * WAVE], f16, tag="sig", name="sig")
                    nc.scalar.activation(sig[:], st["gates"][:], AF.Sigmoid)
                    st["sig"] = sig
                return fn

            def mk_m1(w, t):
                def fn():
                    st = state[w]
                    m1 = apool.tile([128, WAVE], f16, tag="m1", name="m1")
                    nc.vector.scalar_tensor_tensor(
                        m1[:], st["sig"][:, 3 * WAVE : 4 * WAVE], 0.5,
                        st["sig"][:, 0:WAVE], OP.subtract, OP.mult,
                    )
                    st["m1"] = m1
                return fn

            def mk_m2(w, t):
                def fn():
                    st = state[w]
                    m2 = apool.tile([128, WAVE], f16, tag="m2", name="m2")
                    nc.vector.tensor_tensor(
                        m2[:], st["sig"][:, WAVE : 2 * WAVE], st["c_prev"][:], OP.mult
                    )
                    st["m2"] = m2
                return fn

            def mk_c(w, t):
                def fn():
                    st = state[w]
                    c_new = hcpool.tile([128, WAVE], f16, tag="c", name="c")
                    if t == 0:
                        nc.vector.tensor_scalar_mul(c_new[:], st["m1"][:], 2.0)
                        st["h_all"] = opool.tile(
                            [128, T * WAVE], f16, tag="hall", name="hall"
                        )
                    else:
                        nc.vector.scalar_tensor_tensor(
                            c_new[:], st["m1"][:], 2.0, st["m2"][:], OP.mult, OP.add
                        )
                    st["c_prev"] = c_new
                return fn

            def mk_tanh(w, t):
                def fn():
                    st = state[w]
                    tanhc = apool.tile([128, WAVE], f16, tag="tanhc", name="tanhc")
                    nc.scalar.activation(tanhc[:], st["c_prev"][:], AF.Tanh)
                    st["tanhc"] = tanhc
                return fn

            def mk_h(w, t):
                def fn():
                    st = state[w]
                    h_new = st["h_all"][:, ts(t, WAVE)]
                    nc.vector.tensor_tensor(
                        h_new[:], st["sig"][:, 2 * WAVE : 3 * WAVE], st["tanhc"][:],
                        OP.mult,
                    )
                    st["h_new"] = h_new
                return fn

            def mk_dh(w, t):
                def fn():
                    st = state[w]
                    dh = apool.tile([128, WAVE], f16, tag="dh", name="dh")
                    nc.vector.tensor_tensor(
                        dh[:], st["h_new"][:], st["h_prev"][:], OP.subtract
                    )
                    st["dh"] = dh
                    st["h_prev"] = st["h_new"]
                return fn

            def mk_mm(w, t):
                def fn():
                    st = state[w]
                    mv = st["h_new"] if t == 0 else st["dh"]
                    for g in range(4):
                        nc.tensor.matmul(
                            st["gates"][:, ts(g, WAVE)], whht_s[:, ts(g, 128)],
                            mv[:],
                            start=False, stop=(t == T - 2), skip_group_check=True,
                        )
                    if t == 0:
                        st["h_prev"] = st["h_new"]
                return fn

            # rel_pos: h-stationary matmuls (batch on out partitions, N=2)
            def mk_rel_mm(w, e0):
                def fn():
                    st = state[w]
                    blk = e0 // T
                    if e0 % T == 0:
                        st["relp%d" % blk] = scrpool.tile(
                            [128, 2 * T], f32, tag="scratch", name="relp"
                        )
                    rp = st["relp%d" % blk]
                    for e in range(e0, e0 + 6):
                        t = e % T
                        nc.tensor.matmul(
                            rp[:, 2 * t : 2 * t + 2],
                            st["h_all"][:, t * WAVE + blk * 128 :
                                         t * WAVE + blk * 128 + 128],
                            whpt_s[:], start=True, stop=True, skip_group_check=True,
                        )
                return fn

            def mk_rel_copy(w):
                def fn():
                    st = state[w]
                    sb = opool.tile([128, 4 * T], f32, tag="predsb", name="predsb")
                    nc.scalar.copy(sb[:, 0 : 2 * T], st["relp0"][:])
                    nc.vector.tensor_copy(sb[:, 2 * T : 4 * T], st["relp1"][:])
                    st["predsb"] = sb
                return fn

            def mk_rel_dma(w):
                def fn():
                    st = state[w]
                    nc.sync.dma_start(pred[w], st["predsb"][:])
                return fn

            LW = [4600] * NW
            for w in range(NW):
                S = w * WS
                L = LW[w]
                o = lambda x: x * L // 4600
                ev(S - 8000, mk_open(w))
                for g in range(4):
                    ev(S - 7700 + 800 * g, mk_zxp(w, g))
                ev(S - 3700, mk_init_wes(w))
                ev(S - 2600, mk_init_hh(w))
                for t in range(T):
                    tau = S + t * L
                    ev(tau, mk_sig(w, t))
                    if t == 0:
                        ev(tau + o(2700), mk_t0rm(w))
                    ev(tau + o(1250), mk_m1(w, t))
                    if t > 0:
                        ev(tau + o(1700), mk_m2(w, t))
                    ev(tau + o(2150), mk_c(w, t))
                    ev(tau + o(2550), mk_tanh(w, t))
                    ev(tau + o(3100), mk_h(w, t))
                    if t == 0:
                        ev(tau + o(3450), mk_mm(w, t))
                    elif t < T - 1:
                        ev(tau + o(3450), mk_dh(w, t))
                        ev(tau + o(3950), mk_mm(w, t))
                R = S + 11 * L
                for c in range(4):
                    ev(R + 3300 + 600 * c, mk_rel_mm(w, 6 * c))
                ev(R + 5700, mk_rel_copy(w))
                ev(R + 6100, mk_rel_dma(w))
            for p in range(NP):
                E = 2 * p * WS
                # pair 0 runs in the preamble, shifted earlier so wave 0's
                # psi/open chain is ready by S=0
                M0 = -34600 if p == 0 else E - 31100
                ev(-38500 if p == 0 else E - 31900, mk_zxw(p))
                for j in range(8):
                    u = M0 + 2400 * j
                    for ci, k0 in enumerate((0, 3, 6)):
                        ev(u + 750 * ci, mk_mlp1_mm(p, j, k0))
                    if j % 2 == 1:
                        ev(u + 2250, mk_mlp1_ts(p, j - 1))
                        ev(u + 2400, mk_mlp1_ts(p, j))
                u2 = M0 + 7 * 2400 + 2300
                for ci, (j0, nj) in enumerate(((0, 3), (3, 3), (6, 2))):
                    ev(u2 + 600 * ci, mk_mlp2_mm(p, j0, nj))
                ev(u2 + 1800, mk_mlp2_ts(p))
            for jh in range(4):
                ev(-38400 + 100 * jh, mk_w1t_dma(jh))
            ev(-38000, mk_wzxt_dma(0))
            ev(-37900, mk_wzxt_dma(1))
            ev(-37800, mk_w2t_dma())

            def zzm_dma():
                nc.gpsimd.dma_start(zzm_s[:], zzm[:])
            ev(-37000, zzm_dma)

            for _, _, fn in sorted(events, key=lambda e: (e[0], e[1])):
                fn()

    nc.compile()
    return nc


def _prep(inputs):
    """Host-side weight folding + layout prep. Returns per-core input maps."""
    f = np.float64
    W_ih = np.asarray(inputs["W_ih"], f)
    W_hh = np.asarray(inputs["W_hh"], f)
    b_ih = np.asarray(inputs["b_ih"], f)
    b_hh = np.asarray(inputs["b_hh"], f)
    W1 = np.asarray(inputs["W1"], f)
    b1 = np.asarray(inputs["b1"], f)
    W2 = np.asarray(inputs["W2"], f)
    b2 = np.asarray(inputs["b2"], f)
    W_sp = np.asarray(inputs["W_sp"], f)
    b_sp = np.asarray(inputs["b_sp"], f)
    W_hp = np.asarray(inputs["W_hp"], f)
    b_hp = np.asarray(inputs["b_hp"], f)

    W_zx = W_ih[:, :ZX]
    W_emb = W_ih[:, ZX:]
    W_es = W_emb @ W_sp                       # [4H, 2]
    W_hh_f = W_hh + W_es @ W_hp               # [4H, H]
    bias1 = b_ih + b_hh + W_emb @ b_sp + W_es @ b_hp

    # reorder pytorch gates (i, f, g, o) -> (i, f, o, g)
    perm = np.r_[0:H, H : 2 * H, 3 * H : 4 * H, 2 * H : 3 * H]
    W_zx = W_zx[perm]
    W_hh_f = W_hh_f[perm]
    W_es = W_es[perm]
    bias1 = bias1[perm]
    # double the g-gate block: its bank then holds 2*g_pre, so
    # tanh(g) = 2*sigmoid(2*g_pre) - 1 comes out of the one big sigmoid
    dbl = np.ones((G4, 1))
    dbl[3 * H :] = 2.0
    W_zx = W_zx * dbl
    W_hh_f = W_hh_f * dbl
    W_es = W_es * dbl
    bias1 = bias1 * dbl[:, 0]

    def kxm(Wt):
        # [ZX, M] -> [128, KT, M] fp16: 8 full K-tiles + the K=32 tail
        # replicated across the 4 partition row-groups (for tile_position
        # row packing)
        K, M = Wt.shape
        out = np.zeros((KP, M), f)
        out[: 8 * 128] = Wt[: 8 * 128]
        out[8 * 128 :] = np.tile(Wt[8 * 128 :], (4, 1)) * 0.25
        return np.ascontiguousarray(
            out.reshape(KT, 128, M).transpose(1, 0, 2)
        ).astype(np.float16)

    # K=66 bank-open stationary: rows 0-1 bias1 (indicator-selected),
    # rows 2+32*gp+q = the z-column weights W_zx[(2bk+gp)*128+m, 1024+q]
    lhsT66 = np.zeros((66, 2, 128), f)
    b2d = bias1.reshape(4, 128)
    for bk in range(2):
        for r in range(2):
            lhsT66[r, bk] = b2d[2 * bk + r]
        for gp in range(2):
            g = 2 * bk + gp
            lhsT66[2 + 32 * gp : 34 + 32 * gp, bk] = (
                W_zx[g * 128 : (g + 1) * 128, 1024:1056].T
            )

    consts = {
        "w1t": kxm(W1.T),
        "wzxt": kxm(W_zx.T),
        "w2t": np.ascontiguousarray(
            W2.T.reshape(8, 128, H).transpose(1, 0, 2)
        ).astype(np.float16),
        "whht": np.ascontiguousarray(W_hh_f.T).astype(np.float16),
        "whpt": np.ascontiguousarray(W_hp.T).astype(np.float16),
        "whh0": np.ascontiguousarray((W_hh[perm] * dbl).T).astype(np.float16),
        "whhn": np.ascontiguousarray(-(W_hh[perm] * dbl).T).astype(np.float16),
        "lhsT66": np.ascontiguousarray(lhsT66.reshape(66, 256)).astype(np.float16),
        "wes2": np.ascontiguousarray(W_es.T).astype(np.float16),
        "k32": np.ascontiguousarray(-W_es.T).astype(np.float16),
        "b1": np.ascontiguousarray(b1.reshape(8, 128).T).astype(np.float32),
        "b2": b2.reshape(128, 1).astype(np.float32),
    }

    enc = np.asarray(inputs["enc_h_feat"], np.float32)
    z = np.asarray(inputs["z"], np.float32)
    lpr = np.asarray(inputs["last_pos_rel"], np.float32)
    zxT = np.zeros((KP, B), np.float16)
    zxT[:MLP] = enc.T
    zxT[MLP:ZX] = z.T
    # replicate the K=32 tail rows (zx rows 1024..1055) into all 4
    # row-groups of the last K-tile
    zxT[8 * 128 :] = np.tile(zxT[8 * 128 : 8 * 128 + 32], (4, 1))
    lprT = np.ascontiguousarray((lpr - b_hp[None, :]).T).astype(np.float16)

    zT = z.T.astype(f)  # [32, B]
    in_maps = []
    for c in range(NCORES):
        s = slice(c * BC, (c + 1) * BC)
        m = dict(consts)
        m["zxT"] = np.ascontiguousarray(zxT[:, s])
        m["lprT"] = np.ascontiguousarray(lprT[:, s])
        zzm = np.zeros((66, NW, 512), f)
        zzm[0, :, 0:WAVE] = 1.0
        zzm[1, :, WAVE:] = 1.0
        for w in range(NW):
            zw = zT[:, c * BC + w * WAVE : c * BC + (w + 1) * WAVE]
            zzm[2:34, w, 0:WAVE] = zw
            zzm[34:66, w, WAVE:] = zw
        m["zzm"] = np.ascontiguousarray(zzm.reshape(66, NW * 512)).astype(np.float16)
        in_maps.append(m)
    return in_maps


def run(inputs, trace=False):
    from concourse.bass_utils import run_bass_kernel_spmd

    if "nc" not in _cache:
        _cache["nc"] = _build_nc()
    in_maps = _prep(inputs)
    res = run_bass_kernel_spmd(
        _cache["nc"], in_maps, core_ids=list(range(NCORES)), trace=trace
    )
    # per core: [NW, 128, (blk, t, j)]; batch = w*256 + blk*128 + p
    def decode(a):
        a = a.reshape(NW, 128, 2, T, 2)              # w p blk t j
        return a.transpose(0, 2, 1, 3, 4).reshape(BC, T, 2)
    pred = np.concatenate(
        [decode(r["pred"]) for r in res.results], axis=0
    )  # [B, T, 2]
    out = pred.transpose(1, 0, 2) + np.asarray(inputs["b_hp"], np.float32)[None, None, :]
    return np.ascontiguousarray(out), res


def kernel(**inputs) -> np.ndarray:
    out, _ = run(inputs, trace=False)
    return out


# revision 18
# speedup vs baseline: 1.0557x; 1.0557x over previous
"""Trainium2 Bass kernel for the trajectory-decoder LSTM problem.

Math (mirrors the reference, with algebraic folds):
  dec_inp feeds gates only through W_emb; W_sp/W_emb/W_hp collapse:
    W_es = W_emb @ W_sp            [4H, 2]
    gates_t = zx@W_zx.T + bias + r_{t-1}@W_es.T + h_{t-1}@W_hh.T
  For t>=1, r_{t-1} = h_{t-1}@W_hp.T + b_hp, so with
    W_hh' = W_hh + W_es @ W_hp,  bias1 = b_ih + b_hh + W_emb@b_sp + W_es@b_hp
  every step becomes uniform:  gates_t = zx@W_zx.T + bias1 + h_{t-1}@W_hh'.T
  Step 0, rewritten against the SAME stationary W_hh' (so the recurrence
  weights are reused):
    gates_0 = zxp + bias1 + h_init@W_hh'.T + psi@W_es.T
    psi = (lpr - b_hp) - h_init@W_hp.T            [B, 2]
  and after sigmoid_0 the psi term is removed again (gates_1.. use only
  dh@W_hh'.T deltas on the resident PSUM accumulator).
  `last_pos` is dead code (output is just the stacked rel_pos).

Device strategy (pure data-parallel over 8 cores, 4096 batch each):
  - batch on the free dim, features on partitions
  - per 256-batch wave, the 4 gate pre-activations live RESIDENT in one
    2-bank PSUM tile; each step the PE accumulates (h_t - h_{t-1}) @ W_hh'.T
    into it (start=False).  The two bank-open matmuls are K=6: rows 0-1
    inject bias1 via a 0/1 indicator, rows 2-5 inject psi@W_es.T via
    sel-masked psi rows (psi replicated into partition rows 2-5 of the
    moving tile by two tiny SBUF->SBUF DMAs).  psi itself is computed by
    the PE (h_init @ -W_hp.T accumulated with I2 @ (lpr-b_hp)) into the
    wave's own (not yet opened) gates bank.
  - contraction K=1056 exact (no 1152 pad): 8 full K-tiles plus one K=32
    tile evaluated with row-group packing (tile_position): 4 gate tiles
    concurrently for zxp, j-tile pairs for mlp1.  The K=32 zx rows are
    replicated across the 4 partition row-groups host-side.
  - the g-gate weights are doubled on host so ONE sigmoid over all 4 banks
    yields sig(i),sig(f),sig(o),sig(2g); tanh(g)=2*sig(2g)-1 folds into the
    DVE scalar_tensor_tensor ops for m1 and c.
  - per-step elementwise: sigmoid (ACT), then m1/m2/c and h/dh as
    back-to-back DVE ops; tanh on ACT.
  - rel_pos: h-stationary matmuls (batch on out partitions, N=2) into
    small scratch PSUM tiles, copied out via ACT+DVE, one contiguous
    partition-major DMA per wave.
  - waves are software-pipelined on a virtual timeline: every op gets
    an estimated execution time tau and ops are emitted sorted by tau,
    so each engine's in-order queue matches true readiness order.
    Chains step every L=4.6us; waves start every WS=22.4us (PE-
    feasibility bound).  Filler matmuls (mlp/init) are chopped into
    <=0.7us chunks; weight DMAs go on the idle GpSimd queue.
  - all matmul operands fp16 (full PE rate), PSUM accumulation fp32;
    c kept in fp16, b_hp added on host after gather.
"""

import numpy as np

B = 32768
NCORES = 8
BC = B // NCORES          # 4096 batch per core
WAVE = 256                # batch per recurrence chain (2 PSUM banks of gates)
NW = BC // WAVE           # 16 waves
PAIR = 2 * WAVE           # phase-A (mlp) runs at N=512 across wave pairs
T = 12                    # decode steps
H = 128
G4 = 4 * H                # 512 gate features
ZX = 1056
KT = 9                    # 8 full K-tiles + one packed K=32 tile
KP = KT * 128
MLP = 1024
EMB = 64

_cache = {}


def _build_nc():
    import concourse.bass as bass
    import concourse.bacc as bacc
    import concourse.mybir as mybir
    import concourse.tile as tile
    from concourse.bass import ts

    f16 = mybir.dt.float16
    f32 = mybir.dt.float32
    AF = mybir.ActivationFunctionType
    OP = mybir.AluOpType

    nc = bacc.Bacc("TRN2", target_bir_lowering=False)

    zxT = nc.dram_tensor("zxT", [KP, BC], f16, kind="ExternalInput")
    lprT = nc.dram_tensor("lprT", [2, BC], f16, kind="ExternalInput")
    w1t = nc.dram_tensor("w1t", [128, KT, MLP], f16, kind="ExternalInput")
    wzxt = nc.dram_tensor("wzxt", [128, KT, G4], f16, kind="ExternalInput")
    w2t = nc.dram_tensor("w2t", [128, 8, H], f16, kind="ExternalInput")
    whht = nc.dram_tensor("whht", [128, G4], f16, kind="ExternalInput")
    whpt = nc.dram_tensor("whpt", [128, 2], f16, kind="ExternalInput")
    whh0 = nc.dram_tensor("whh0", [128, G4], f16, kind="ExternalInput")
    whhn = nc.dram_tensor("whhn", [128, G4], f16, kind="ExternalInput")
    lhsT66 = nc.dram_tensor("lhsT66", [66, 2 * 128], f16, kind="ExternalInput")
    zzm = nc.dram_tensor("zzm", [66, NW * 512], f16, kind="ExternalInput")
    wes2 = nc.dram_tensor("wes2", [2, G4], f16, kind="ExternalInput")
    k32 = nc.dram_tensor("k32", [2, G4], f16, kind="ExternalInput")
    b1 = nc.dram_tensor("b1", [128, 8], f32, kind="ExternalInput")
    b2 = nc.dram_tensor("b2", [128, 1], f32, kind="ExternalInput")
    # per wave: [partition=batch, (blk, t, j)] — partition-major so the
    # rel output DMA is one clean 192B-per-partition descriptor
    pred = nc.dram_tensor("pred", [NW, 128, 4 * T], f32, kind="ExternalOutput")

    with tile.TileContext(nc) as tc:
        with (
            tc.tile_pool(name="consts", bufs=1) as cpool,
            tc.tile_pool(name="zx", bufs=2) as zxpool,
            tc.tile_pool(name="h1", bufs=2) as h1pool,
            tc.tile_pool(name="hc", bufs=10) as hcpool,
            tc.tile_pool(name="acts", bufs=6) as apool,
            tc.tile_pool(name="outs", bufs=3) as opool,
            tc.tile_pool(name="scrps", bufs=2, space="PSUM") as scrpool,
            tc.tile_pool(name="gateps", bufs=3, space="PSUM") as gatepool,
        ):
            # ---- constants: small ones loaded upfront; the big mlp/zxp
            # weights are DMA'd in chunks as tau-events ----
            w1t_s = cpool.tile([128, KT, MLP], f16)
            wzxt_s = cpool.tile([128, KT, G4], f16)
            w2t_s = cpool.tile([128, 8, H], f16)
            whht_s = cpool.tile([128, G4], f16)
            nc.sync.dma_start(whht_s[:], whht[:])
            whpt_s = cpool.tile([128, 2], f16)
            nc.sync.dma_start(whpt_s[:], whpt[:])
            whh0_s = cpool.tile([128, G4], f16)
            nc.sync.dma_start(whh0_s[:], whh0[:])
            whhn_s = cpool.tile([128, G4], f16)
            nc.sync.dma_start(whhn_s[:], whhn[:])
            lhsT66_s = cpool.tile([66, 2 * 128], f16)
            nc.sync.dma_start(lhsT66_s[:], lhsT66[:])
            zzm_s = cpool.tile([66, NW * 512], f16)
            wes2_s = cpool.tile([2, G4], f16)
            nc.sync.dma_start(wes2_s[:], wes2[:])
            k32_s = cpool.tile([2, G4], f16)
            nc.sync.dma_start(k32_s[:], k32[:])
            b1_s = cpool.tile([128, 8], f32)
            nc.sync.dma_start(b1_s[:], b1[:])
            b2_s = cpool.tile([128, 1], f32)
            nc.sync.dma_start(b2_s[:], b2[:])
            lpr_s = cpool.tile([2, BC], f16)
            nc.sync.dma_start(lpr_s[:], lprT[:])

            zxT_v = zxT.rearrange("(k p) b -> p k b", p=128)

            # Virtual-timeline schedule (see module docstring).
            NP = NW // 2
            state = [dict() for _ in range(NW)]
            pair_state = [dict() for _ in range(NP)]
            events = []

            L = 5000
            WS = 23400

            def ev(tau, fn):
                events.append((tau, len(events), fn))

            def mk_w1t_dma(jh):
                def fn():
                    nc.gpsimd.dma_start(
                        w1t_s[:, :, ts(jh, 256)], w1t[:, :, ts(jh, 256)]
                    )
                return fn

            def mk_wzxt_dma(half):
                def fn():
                    nc.gpsimd.dma_start(
                        wzxt_s[:, :, ts(half, 256)], wzxt[:, :, ts(half, 256)]
                    )
                return fn

            def mk_w2t_dma():
                def fn():
                    nc.sync.dma_start(w2t_s[:], w2t[:])
                return fn

            def mk_zxw(p):
                def fn():
                    st = pair_state[p]
                    zxw = zxpool.tile([128, KT, PAIR], f16, tag="zxw", name="zxw")
                    nc.gpsimd.dma_start(zxw[:], zxT_v[:, :, ts(p, PAIR)])
                    st["zxw"] = zxw
                    st["h1"] = h1pool.tile([128, 8, PAIR], f16, tag="h1", name="h1")
                return fn

            def mk_mlp1_mm(p, j, k0):
                def fn():
                    st = pair_state[p]
                    key = "ps%d" % (j % 2)
                    if k0 == 0:
                        st[key] = scrpool.tile([128, PAIR], f32, tag="scratch", name="ps")
                    for k in range(k0, min(k0 + 3, KT)):
                        nc.tensor.matmul(
                            st[key][:], w1t_s[:, k, ts(j, 128)], st["zxw"][:, k, :],
                            start=(k == 0), stop=(k == KT - 1),
                            skip_group_check=True,
                        )
                return fn

            # K=32 tail tile: j-pair 2-packed on PE row groups 0 / 32
            def mk_mlp1_pk(p, je):
                def fn():
                    st = pair_state[p]
                    nc.tensor.matmul(
                        st["ps0"][:], w1t_s[0:32, 8, ts(je, 128)],
                        st["zxw"][0:32, 8, :],
                        start=False, stop=True, skip_group_check=True,
                    )
                    nc.tensor.matmul(
                        st["ps1"][:], w1t_s[32:64, 8, ts(je + 1, 128)],
                        st["zxw"][32:64, 8, :],
                        start=False, stop=True, skip_group_check=True,
                    )
                return fn

            def mk_mlp1_ts(p, j):
                def fn():
                    st = pair_state[p]
                    key = "ps%d" % (j % 2)
                    nc.vector.tensor_scalar(
                        st["h1"][:, j, :], st[key][:], b1_s[:, j : j + 1],
                        0.0, OP.add, OP.max,
                    )
                return fn

            def mk_mlp2_mm(p, j0, nj):
                def fn():
                    st = pair_state[p]
                    if j0 == 0:
                        st["ps0"] = scrpool.tile([128, PAIR], f32, tag="scratch", name="ps")
                    for j in range(j0, j0 + nj):
                        nc.tensor.matmul(
                            st["ps0"][:], w2t_s[:, j, :], st["h1"][:, j, :],
                            start=(j == 0), stop=(j == 7), skip_group_check=True,
                        )
                return fn

            def mk_mlp2_ts(p):
                def fn():
                    st = pair_state[p]
                    hi = h1pool.tile([128, PAIR], f16, tag="hinit", name="hinit")
                    nc.vector.tensor_scalar(
                        hi[:], st["ps0"][:], b2_s[:, 0:1], 0.0, OP.add, OP.max
                    )
                    st["h_init"] = hi
                return fn

            # bank-open: K=66 matmul injects bias1 (indicator rows 0-1)
            # plus the z-dims' zxp contribution (sel-masked z rows 2-65),
            # and clears has_written for the whole bank
            def mk_open(w):
                def fn():
                    st = state[w]
                    gates = gatepool.tile([128, 4 * WAVE], f32, tag="gates", name="gates")
                    st["gates"] = gates
                    for bk in range(2):
                        nc.tensor.matmul(
                            gates[:, ts(bk, 2 * WAVE)],
                            lhsT66_s[:, ts(bk, 128)], zzm_s[:, ts(w, 512)],
                            start=True, stop=False, skip_group_check=True,
                        )
                return fn

            # gates_0 = zxp + bias + h_init@W_hh.T + (lpr-b_hp)@W_es.T
            def mk_init_wes(w):
                def fn():
                    st = state[w]
                    for g in range(4):
                        nc.tensor.matmul(
                            st["gates"][:, ts(g, WAVE)], wes2_s[:, ts(g, 128)],
                            lpr_s[:, ts(w, WAVE)],
                            start=False, stop=False, skip_group_check=True,
                        )
                return fn

            def mk_init_hh(w):
                def fn():
                    st = state[w]
                    st["h_prev"] = pair_state[w // 2]["h_init"][:, ts(w % 2, WAVE)]
                    for g in range(4):
                        nc.tensor.matmul(
                            st["gates"][:, ts(g, WAVE)], whh0_s[:, ts(g, 128)],
                            st["h_prev"][:],
                            start=False, stop=False, skip_group_check=True,
                        )
                return fn

            # step-0 removal: gates += -h_init@W_hh.T - (lpr-b_hp)@W_es.T
            def mk_t0rm(w):
                def fn():
                    st = state[w]
                    for g in range(4):
                        gp = st["gates"][:, ts(g, WAVE)]
                        nc.tensor.matmul(
                            gp[:], whhn_s[:, ts(g, 128)], st["h_prev"][:],
                            start=False, stop=False, skip_group_check=True,
                        )
                        nc.tensor.matmul(
                            gp[:], k32_s[:, ts(g, 128)], lpr_s[:, ts(w, WAVE)],
                            start=False, stop=False, skip_group_check=True,
                        )
                return fn

            def mk_zxp(w, g):
                def fn():
                    st = state[w]
                    pst = pair_state[w // 2]
                    hs = ts(w % 2, WAVE)
                    gp = st["gates"][:, ts(g, WAVE)]
                    for k in range(8):
                        nc.tensor.matmul(
                            gp[:], wzxt_s[:, k, ts(g, 128)], pst["zxw"][:, k, hs],
                            start=False, stop=False, skip_group_check=True,
                        )
                return fn

            # K=32 tail tile: all 4 gates concurrently on row groups 0..96
            def mk_zxp_pk(w):
                def fn():
                    st = state[w]
                    pst = pair_state[w // 2]
                    hs = ts(w % 2, WAVE)
                    for g in range(4):
                        nc.tensor.matmul(
                            st["gates"][:, ts(g, WAVE)],
                            wzxt_s[32 * g : 32 * g + 32, 8, ts(g, 128)],
                            pst["zxw"][32 * g : 32 * g + 32, 8, hs],
                            start=False, stop=False, skip_group_check=True,
                            tile_position=(32 * g, 0),
                        )
                return fn

            def mk_sig(w, t):
                def fn():
                    st = state[w]
                    sig = apool.tile([128, 4 * WAVE], f16, tag="sig", name="sig")
                    nc.scalar.activation(sig[:], st["gates"][:], AF.Sigmoid)
                    st["sig"] = sig
                return fn

            def mk_m1(w, t):
                def fn():
                    st = state[w]
                    m1 = apool.tile([128, WAVE], f16, tag="m1", name="m1")
                    nc.vector.scalar_tensor_tensor(
                        m1[:], st["sig"][:, 3 * WAVE : 4 * WAVE], 0.5,
                        st["sig"][:, 0:WAVE], OP.subtract, OP.mult,
                    )
                    st["m1"] = m1
                return fn

            def mk_m2(w, t):
                def fn():
                    st = state[w]
                    m2 = apool.tile([128, WAVE], f16, tag="m2", name="m2")
                    nc.vector.tensor_tensor(
                        m2[:], st["sig"][:, WAVE : 2 * WAVE], st["c_prev"][:], OP.mult
                    )
                    st["m2"] = m2
                return fn

            def mk_c(w, t):
                def fn():
                    st = state[w]
                    c_new = hcpool.tile([128, WAVE], f16, tag="c", name="c")
                    if t == 0:
                        nc.vector.tensor_scalar_mul(c_new[:], st["m1"][:], 2.0)
                        st["h_all"] = opool.tile(
                            [128, T * WAVE], f16, tag="hall", name="hall"
                        )
                    else:
                        nc.vector.scalar_tensor_tensor(
                            c_new[:], st["m1"][:], 2.0, st["m2"][:], OP.mult, OP.add
                        )
                    st["c_prev"] = c_new
                return fn

            def mk_tanh(w, t):
                def fn():
                    st = state[w]
                    tanhc = apool.tile([128, WAVE], f16, tag="tanhc", name="tanhc")
                    nc.scalar.activation(tanhc[:], st["c_prev"][:], AF.Tanh)
                    st["tanhc"] = tanhc
                return fn

            def mk_h(w, t):
                def fn():
                    st = state[w]
                    h_new = st["h_all"][:, ts(t, WAVE)]
                    nc.vector.tensor_tensor(
                        h_new[:], st["sig"][:, 2 * WAVE : 3 * WAVE], st["tanhc"][:],
                        OP.mult,
                    )
                    st["h_new"] = h_new
                return fn

            def mk_dh(w, t):
                def fn():
                    st = state[w]
                    dh = apool.tile([128, WAVE], f16, tag="dh", name="dh")
                    nc.vector.tensor_tensor(
                        dh[:], st["h_new"][:], st["h_prev"][:], OP.subtract
                    )
                    st["dh"] = dh
                    st["h_prev"] = st["h_new"]
                return fn

            def mk_mm(w, t):
                def fn():
                    st = state[w]
                    mv = st["h_new"] if t == 0 else st["dh"]
                    for g in range(4):
                        nc.tensor.matmul(
                            st["gates"][:, ts(g, WAVE)], whht_s[:, ts(g, 128)],
                            mv[:],
                            start=False, stop=(t == T - 2), skip_group_check=True,
                        )
                    if t == 0:
                        st["h_prev"] = st["h_new"]
                return fn

            # rel_pos: h-stationary matmuls (batch on out partitions, N=2)
            def mk_rel_mm(w, e0):
                def fn():
                    st = state[w]
                    blk = e0 // T
                    if e0 % T == 0:
                        st["relp%d" % blk] = scrpool.tile(
                            [128, 2 * T], f32, tag="scratch", name="relp"
                        )
                    rp = st["relp%d" % blk]
                    for e in range(e0, e0 + 6):
                        t = e % T
                        nc.tensor.matmul(
                            rp[:, 2 * t : 2 * t + 2],
                            st["h_all"][:, t * WAVE + blk * 128 :
                                         t * WAVE + blk * 128 + 128],
                            whpt_s[:], start=True, stop=True, skip_group_check=True,
                        )
                return fn

            def mk_rel_copy(w):
                def fn():
                    st = state[w]
                    sb = opool.tile([128, 4 * T], f32, tag="predsb", name="predsb")
                    nc.scalar.copy(sb[:, 0 : 2 * T], st["relp0"][:])
                    nc.vector.tensor_copy(sb[:, 2 * T : 4 * T], st["relp1"][:])
                    st["predsb"] = sb
                return fn

            def mk_rel_dma(w):
                def fn():
                    st = state[w]
                    nc.sync.dma_start(pred[w], st["predsb"][:])
                return fn

            for w in range(NW):
                S = w * WS
                ev(S - 8000, mk_open(w))
                for g in range(4):
                    ev(S - 7800 + 900 * g, mk_zxp(w, g))
                ev(S - 3600, mk_init_wes(w))
                ev(S - 2600, mk_init_hh(w))
                for t in range(T):
                    tau = S + t * L
                    ev(tau, mk_sig(w, t))
                    if t == 0:
                        ev(tau + 2700, mk_t0rm(w))
                    ev(tau + 1250, mk_m1(w, t))
                    if t > 0:
                        ev(tau + 1700, mk_m2(w, t))
                    ev(tau + 2150, mk_c(w, t))
                    ev(tau + 2550, mk_tanh(w, t))
                    ev(tau + 3100, mk_h(w, t))
                    if t == 0:
                        ev(tau + 3450, mk_mm(w, t))
                    elif t < T - 1:
                        ev(tau + 3450, mk_dh(w, t))
                        ev(tau + 3950, mk_mm(w, t))
                R = S + 11 * L
                for c in range(4):
                    ev(R + 3300 + 600 * c, mk_rel_mm(w, 6 * c))
                ev(R + 5700, mk_rel_copy(w))
                ev(R + 6100, mk_rel_dma(w))
            for p in range(NP):
                E = 2 * p * WS
                ev(-42000 if p == 0 else E - 27800, mk_zxw(p))
                for j in range(8):
                    u = E - 27000 + 2400 * j
                    for ci, k0 in enumerate((0, 3, 6)):
                        ev(u + 750 * ci, mk_mlp1_mm(p, j, k0))
                    if j % 2 == 1:
                        ev(u + 2250, mk_mlp1_ts(p, j - 1))
                        ev(u + 2400, mk_mlp1_ts(p, j))
                for ci, (j0, nj) in enumerate(((0, 3), (3, 3), (6, 2))):
                    ev(E - 7600 + 700 * ci, mk_mlp2_mm(p, j0, nj))
                ev(E - 5800, mk_mlp2_ts(p))
            for jh in range(4):
                ev(-40600 + 300 * jh, mk_w1t_dma(jh))
            ev(-39400, mk_wzxt_dma(0))
            ev(-39100, mk_wzxt_dma(1))
            ev(-38800, mk_w2t_dma())

            def zzm_dma():
                nc.gpsimd.dma_start(zzm_s[:], zzm[:])
            ev(-38700, zzm_dma)

            for _, _, fn in sorted(events, key=lambda e: (e[0], e[1])):
                fn()

    nc.compile()
    return nc


def _prep(inputs):
    """Host-side weight folding + layout prep. Returns per-core input maps."""
    f = np.float64
    W_ih = np.asarray(inputs["W_ih"], f)
    W_hh = np.asarray(inputs["W_hh"], f)
    b_ih = np.asarray(inputs["b_ih"], f)
    b_hh = np.asarray(inputs["b_hh"], f)
    W1 = np.asarray(inputs["W1"], f)
    b1 = np.asarray(inputs["b1"], f)
    W2 = np.asarray(inputs["W2"], f)
    b2 = np.asarray(inputs["b2"], f)
    W_sp = np.asarray(inputs["W_sp"], f)
    b_sp = np.asarray(inputs["b_sp"], f)
    W_hp = np.asarray(inputs["W_hp"], f)
    b_hp = np.asarray(inputs["b_hp"], f)

    W_zx = W_ih[:, :ZX]
    W_emb = W_ih[:, ZX:]
    W_es = W_emb @ W_sp                       # [4H, 2]
    W_hh_f = W_hh + W_es @ W_hp               # [4H, H]
    bias1 = b_ih + b_hh + W_emb @ b_sp + W_es @ b_hp

    # reorder pytorch gates (i, f, g, o) -> (i, f, o, g)
    perm = np.r_[0:H, H : 2 * H, 3 * H : 4 * H, 2 * H : 3 * H]
    W_zx = W_zx[perm]
    W_hh_f = W_hh_f[perm]
    W_es = W_es[perm]
    bias1 = bias1[perm]
    # double the g-gate block: its bank then holds 2*g_pre, so
    # tanh(g) = 2*sigmoid(2*g_pre) - 1 comes out of the one big sigmoid
    dbl = np.ones((G4, 1))
    dbl[3 * H :] = 2.0
    W_zx = W_zx * dbl
    W_hh_f = W_hh_f * dbl
    W_es = W_es * dbl
    bias1 = bias1 * dbl[:, 0]

    def kxm(Wt):
        # [ZX, M] -> [128, KT, M] fp16: 8 full K-tiles + the K=32 tail
        # replicated across the 4 partition row-groups (for tile_position
        # row packing)
        K, M = Wt.shape
        out = np.zeros((KP, M), f)
        out[: 8 * 128] = Wt[: 8 * 128]
        out[8 * 128 :] = np.tile(Wt[8 * 128 :], (4, 1)) * 0.25
        return np.ascontiguousarray(
            out.reshape(KT, 128, M).transpose(1, 0, 2)
        ).astype(np.float16)

    # K=66 bank-open stationary: rows 0-1 bias1 (indicator-selected),
    # rows 2+32*gp+q = the z-column weights W_zx[(2bk+gp)*128+m, 1024+q]
    lhsT66 = np.zeros((66, 2, 128), f)
    b2d = bias1.reshape(4, 128)
    for bk in range(2):
        for r in range(2):
            lhsT66[r, bk] = b2d[2 * bk + r]
        for gp in range(2):
            g = 2 * bk + gp
            lhsT66[2 + 32 * gp : 34 + 32 * gp, bk] = (
                W_zx[g * 128 : (g + 1) * 128, 1024:1056].T
            )

    consts = {
        "w1t": kxm(W1.T),
        "wzxt": kxm(W_zx.T),
        "w2t": np.ascontiguousarray(
            W2.T.reshape(8, 128, H).transpose(1, 0, 2)
        ).astype(np.float16),
        "whht": np.ascontiguousarray(W_hh_f.T).astype(np.float16),
        "whpt": np.ascontiguousarray(W_hp.T).astype(np.float16),
        "whh0": np.ascontiguousarray((W_hh[perm] * dbl).T).astype(np.float16),
        "whhn": np.ascontiguousarray(-(W_hh[perm] * dbl).T).astype(np.float16),
        "lhsT66": np.ascontiguousarray(lhsT66.reshape(66, 256)).astype(np.float16),
        "wes2": np.ascontiguousarray(W_es.T).astype(np.float16),
        "k32": np.ascontiguousarray(-W_es.T).astype(np.float16),
        "b1": np.ascontiguousarray(b1.reshape(8, 128).T).astype(np.float32),
        "b2": b2.reshape(128, 1).astype(np.float32),
    }

    enc = np.asarray(inputs["enc_h_feat"], np.float32)
    z = np.asarray(inputs["z"], np.float32)
    lpr = np.asarray(inputs["last_pos_rel"], np.float32)
    zxT = np.zeros((KP, B), np.float16)
    zxT[:MLP] = enc.T
    zxT[MLP:ZX] = z.T
    # replicate the K=32 tail rows (zx rows 1024..1055) into all 4
    # row-groups of the last K-tile
    zxT[8 * 128 :] = np.tile(zxT[8 * 128 : 8 * 128 + 32], (4, 1))
    lprT = np.ascontiguousarray((lpr - b_hp[None, :]).T).astype(np.float16)

    zT = z.T.astype(f)  # [32, B]
    in_maps = []
    for c in range(NCORES):
        s = slice(c * BC, (c + 1) * BC)
        m = dict(consts)
        m["zxT"] = np.ascontiguousarray(zxT[:, s])
        m["lprT"] = np.ascontiguousarray(lprT[:, s])
        zzm = np.zeros((66, NW, 512), f)
        zzm[0, :, 0:WAVE] = 1.0
        zzm[1, :, WAVE:] = 1.0
        for w in range(NW):
            zw = zT[:, c * BC + w * WAVE : c * BC + (w + 1) * WAVE]
            zzm[2:34, w, 0:WAVE] = zw
            zzm[34:66, w, WAVE:] = zw
        m["zzm"] = np.ascontiguousarray(zzm.reshape(66, NW * 512)).astype(np.float16)
        in_maps.append(m)
    return in_maps


def run(inputs, trace=False):
    from concourse.bass_utils import run_bass_kernel_spmd

    if "nc" not in _cache:
        _cache["nc"] = _build_nc()
    in_maps = _prep(inputs)
    res = run_bass_kernel_spmd(
        _cache["nc"], in_maps, core_ids=list(range(NCORES)), trace=trace
    )
    # per core: [NW, 128, (blk, t, j)]; batch = w*256 + blk*128 + p
    def decode(a):
        a = a.reshape(NW, 128, 2, T, 2)              # w p blk t j
        return a.transpose(0, 2, 1, 3, 4).reshape(BC, T, 2)
    pred = np.concatenate(
        [decode(r["pred"]) for r in res.results], axis=0
    )  # [B, T, 2]
    out = pred.transpose(1, 0, 2) + np.asarray(inputs["b_hp"], np.float32)[None, None, :]
    return np.ascontiguousarray(out), res


def kernel(**inputs) -> np.ndarray:
    out, _ = run(inputs, trace=False)
    return out
